# revision 4
# baseline (speedup 1.0000x reference)
"""Trainium2 distributed kernel for the modular spiking-network module.

Model (reference semantics):
  1. 16 modular units, each a LIF recurrence over shared input spikes
     (T=100, N=1024) with per-unit input / recurrent [N,N] weights.
  2. Per-unit mean activity -> coordinator MLP -> sigmoid probs [16,16].
  3. Bernoulli routing matrix conn = (U42 < probs), U42 fixed uniform draws.
  4. routed = einsum('ij,itn->tjn', conn, outputs);
     applied = einsum('tjn,jnm->tjm', routed, unit_w);
     out = applied.mean(axis=1) + 1.5 * input_spikes.

Key structural facts exploited on-device:

  (a) While no neuron has spiked, the LIF dynamics are LINEAR, so the
      membrane drift is a causal filter of the input currents:
      v_dec(t) - V_LEAK = (K @ sp @ w_in[u].T)[t], with K a constant
      [T,T] kernel.  Folding Ksp = K @ sp on the host (tiny [T,N]
      precompute), the whole per-unit trajectory is ONE matmul
      drift_u = Ksp @ w_in[u].T, and a spike exists iff
      max(drift_u) > V_TH - V_LEAK = 15.

  (b) If no spike fires in ANY unit, every downstream stage is exactly
      zero: outputs == 0 -> activity == 0 -> routed == 0 -> applied == 0,
      and the module output is exactly DIRECT_WEIGHT * input_spikes.

So the device does exactly the irreducible work: it streams all 16
units' [N,N] input weights (fp8, units sharded 2-per-core), computes
each unit's full drift trajectory, and reduces it to a per-core
max-drift scalar.  Each core also emits its 128-column shard of the
residual output 1.5 * input_spikes.  The host ORs the 8 max-drift flags;
if any unit would spike (never observed with the benchmark weight
scales: max drift ~0.14 vs a 15.0 threshold gap, and we flag at 7.5 to
absorb fp8 rounding), it falls back to an exact sequential evaluation.
"""

import ml_dtypes
import numpy as np

import concourse.mybir as mybir
import concourse.tile as tile
from concourse import bacc
from concourse.bass_utils import run_bass_kernel_spmd

# ---------------------------------------------------------------- constants
T, U, N, H = 100, 16, 1024, 128
NCORES = 8
UPC = U // NCORES          # units per core
KC = N // 128              # 128-row chunks per [N,N] matrix
DT = 1e-3
TAU_MEM_INV = 1.0 / 20.0
TAU_SYN_INV = 1.0 / 10.0
V_LEAK = -70.0
V_TH = -55.0
V_RESET = -70.0
DIRECT_WEIGHT = 1.5
THRESH = np.float32(V_TH - V_LEAK)       # 15.0

# fp8 scaling: keep both operands well inside e4m3's normal range so the
# detection matmul loses no low-magnitude rows to denormal flush.
KSP_SCALE = 64.0           # Ksp entries <= ~0.25  -> <= 16
W_SCALE = 16.0             # |w_in| <= 1/32        -> <= 0.5
# device drift is scaled by KSP_SCALE*W_SCALE; flag at half the true
# threshold so fp8 rounding can never hide a real crossing.
DET_TH = 0.5 * float(THRESH) * KSP_SCALE * W_SCALE   # 7680.0

# jax.random.uniform(jax.random.key(42), (16, 16)) — fixed module constant
# (used only by the exact host fallback).
_U42_HEX = (
    "d010183f4043e03ee8e9203f80a1013c80838e3eb0304c3f7c265a3f6868763f"
    "5c332d3f406be33dc8f2fd3e3c2c3b3f9042423e40201e3ea0c31b3dccd2ab3e"
    "3ea26d3f10c61c3f5039fc3eb6da3b3fe0bf413e108f1d3e201c183d049aa83e"
    "86ec6f3f2883183fe855ec3eca682b3fc0c18f3d04c5c93e36d00e3f5c50b53e"
    "62a8733f4a50223f00bcd53bc4c28c3ebcd8493f20dc563f66c5683f1ae2143f"
    "c434d93e56781c3f6890fb3e40573e3f50b3653ea01e5c3ec0142f3e80e2933d"
    "b015c23e38f8073ffc2e943e36f2513fec45663f4c29093f00d2af3eae696a3f"
    "7478113f48c5ce3ee8550b3fd06da63e8afa633fa27d023f1ce2823e0e66473f"
    "12b04b3f06e0533ffa44633f1610023f7c59813e94e9453ff6614f3fe66d5a3f"
    "a07c703fc464243f2093423d3062b93ede997d3f3ec83f3f00ec6d3e1890483e"
    "486f023e409cbc3c7c069e3eb8cf5a3faa59713f540c263fe09f053de8e2a93e"
    "a0ae6d3f661b1f3fa4ddf43ea687303f386d153ea0886f3df403b33eb271773f"
    "d89e2a3f605a8a3d70fbcd3e5892093f5082ae3e9ee66a3f6c93113fe00ace3e"
    "f82d0a3fe0f2a03edcde643f32980d3f8012be3eec2a7a3fa66f303fc8a8133e"
    "a0005e3dacdcbf3ea6cd7b3f808a333fc80d0c3e800b053c8068803e5673443f"
    "d8f04c3f36e95d3fcedb7f3fd6b93b3ff8e34d3e48d70b3e005bf73bd0be8f3e"
    "d2404c3fac81543f6601653fb60e063fe036803eaa2e4c3ffe46543f7c98643f"
    "343e053f8c9e8c3ed4af403fdc624d3fc0f9563f8ac5613fb4a30f3f5cdda63e"
    "befb6a3ff4d4193fc46fff3e700f333f18ff293e7091c43d6c93fa3efaf6363f"
    "d853063ee017753d30b0a53eeec4693f70fd1f3fd029e73e0c792b3fc017d53d"
    "dc2ff23e96723e3f88e2423e1892343e5000b03d8490c03e78de0c3f24efab3e"
    "daa8673fbc12033fb036943e70f7583f8e2e7d3f9e9b363f98d2073ec051723d"
    "8c0ea53edac4693f964e1f3f748ae43e6853283f50c6e33d1c55e83e1481243f"
    "40e3ba3c48f88e3e2a37423f6e9e483f24cd5d3ffc6c773f9a2a223f20880a3d"
    "345bbb3e4ca0773f40b9233fa05b383dfcc6b73e2a2f7b3f00b03a3f182a663e"
    "98c67f3ed01d4c3eb8af2b3e50adc93d9c6bfd3e3ccb313f50d83c3ec069963d"
    "507bd23ec8de1e3f9c3de23e96912e3fc07e8c3d006adf3e54c0133f9c4fd63e"
    "f6f71a3f40aff23e7c163e3f204d423ec06c363e806bbc3d485ec73ee2e90b3f"
    "e8cdb63e507e7a3f024d383f28d4703ec0e9533ed892153e8008cb3c044c803e"
    "deda4c3ff020553f80d2663f3a39013f08db9d3e2246513f00166e3fbeb6103f"
    "30f7db3eba7f173fc4eec43e3e65083fa886b83ee008743f029f243fc06eb63c"
    "68da8f3e4a5e433f3c384a3f6af7583f426b7d3f8c54363f78ae003ec0634d3d"
)


def _u42() -> np.ndarray:
    return np.frombuffer(bytes.fromhex(_U42_HEX), dtype=np.float32).reshape(U, U)


def _kmat() -> np.ndarray:
    """K[t, r] = m * (a^(t-r) - b^(t-r)) / (a - b) for r < t, else 0 (f32)."""
    m = DT * TAU_MEM_INV
    a = 1.0 - m
    b = 1.0 - DT * TAU_SYN_INV
    d = np.arange(T, dtype=np.float64)
    coef = np.zeros(T, np.float64)
    coef[1:] = m * (a ** d[1:] - b ** d[1:]) / (a - b)
    idx = np.arange(T)
    K = coef[np.clip(idx[:, None] - idx[None, :], 0, T - 1)]
    K[idx[:, None] <= idx[None, :]] = 0.0
    return K.astype(np.float32)


# ---------------------------------------------------------------- graph
_GRAPH_CACHE = {}


def _build_graph():
    if "nc" in _GRAPH_CACHE:
        return _GRAPH_CACHE["nc"]

    f32 = mybir.dt.float32
    fp8 = mybir.dt.float8e4
    Alu = mybir.AluOpType

    nc = bacc.Bacc("TRN2", target_bir_lowering=False, debug=False,
                   num_devices=NCORES)

    # I/O (per-core shards / replicas)
    kspt_ext = nc.dram_tensor("kspt", [128, KC * 128], fp8,
                              kind="ExternalInput").ap()
    wint_ext = nc.dram_tensor("wint", [128, UPC * KC * N], fp8,
                              kind="ExternalInput").ap()
    spc_ext = nc.dram_tensor("spc", [T, 128], f32, kind="ExternalInput").ap()
    out_ext = nc.dram_tensor("out", [T, 128], f32, kind="ExternalOutput").ap()
    zsum_ext = nc.dram_tensor("zsum", [1, 1], f32, kind="ExternalOutput").ap()

    with tile.TileContext(nc) as tc:
        with (
            tc.tile_pool(name="wpool", bufs=4) as wpool,      # weight chunks
            tc.tile_pool(name="work", bufs=1) as work,        # persistents
            tc.tile_pool(name="ps", bufs=4, space="PSUM") as ps,
        ):
            # ---------- input DMAs, issue spread across all 5 engine
            # sequencers (a dma_start costs ~0.8us of issue time on its
            # engine; serializing them on Sync was the v2 bottleneck).
            kspt = work.tile([128, KC, 128], fp8)
            nc.sync.dma_start(kspt[:],
                              kspt_ext.rearrange("p (k t) -> p k t", t=128))

            # both units' [N,N] transposed input weights (fp8), as 4 big
            # chunks of 4 k-tiles each (4KB/partition rows).  Only Sync,
            # Activation and GpSimd can initiate DMAs.
            wch = {}
            issuers = [nc.sync, nc.scalar, nc.gpsimd, nc.gpsimd]
            for u in range(UPC):
                for h in range(2):
                    w = wpool.tile([128, 4, N], fp8, tag="wchunk",
                                   name=f"win_{u}_{h}")
                    issuers[2 * u + h].dma_start(
                        w[:], wint_ext[:, (u * KC + 4 * h) * N:
                                       (u * KC + 4 * h + 4) * N]
                        .rearrange("p (k n) -> p k n", k=4))
                    wch[(u, h)] = w

            spc = work.tile([T, 128], f32)
            nc.scalar.dma_start(spc[:], spc_ext)

            # ---------- residual output shard: 1.5 * spikes (independent)
            outc = work.tile([T, 128], f32)
            nc.vector.tensor_scalar(outc[:], spc[:], DIRECT_WEIGHT, None,
                                    Alu.mult)
            nc.scalar.dma_start(out_ext, outc[:])

            # ---------- spike detection: drift_u = Ksp @ w_in[u].T
            # one fp8 DoubleRow matmul chain per (unit, 512-col half);
            # reduce each PSUM tile to a per-timestep max as it completes.
            mxc = work.tile([128, UPC * 2], f32)
            for u in range(UPC):
                for mh in range(2):
                    pv = ps.tile([128, 512], f32, tag="ps",
                                 name=f"pv_{u}_{mh}")
                    for jp in range(KC // 2):
                        h, kk = divmod(jp, 2)
                        nc.tensor.matmul(
                            pv[:, :],
                            kspt[:, 2 * jp:2 * jp + 2, :],
                            wch[(u, h)][:, 2 * kk:2 * kk + 2,
                                        mh * 512:(mh + 1) * 512],
                            start=(jp == 0), stop=(jp == KC // 2 - 1),
                            perf_mode=mybir.MatmulPerfMode.DoubleRow)
                    g = 2 * u + mh
                    nc.vector.tensor_reduce(mxc[:, g:g + 1], pv[:],
                                            mybir.AxisListType.X, Alu.max)

            # cross-partition max -> [1,1] scalar drift flag for the host;
            # reduce + store both live on GpSimd so the tail is short.
            zs = work.tile([1, 1], f32)
            nc.gpsimd.tensor_reduce(zs[:], mxc[:], mybir.AxisListType.XYZWC,
                                    Alu.max)
            nc.gpsimd.dma_start(zsum_ext, zs[:])

    nc.compile()
    _GRAPH_CACHE["nc"] = nc
    return nc


# ---------------------------------------------------------------- host prep
def _prep_in_maps(sp, w_in):
    K32 = _kmat()
    ksp = (K32.astype(np.float64) @ sp.astype(np.float64)) * KSP_SCALE  # [T,N]
    kspt3 = np.zeros((128, KC, 128), np.float32)
    kspt3[:, :, :T] = ksp.T.reshape(KC, 128, T).transpose(1, 0, 2)
    kspt = np.ascontiguousarray(
        kspt3.reshape(128, KC * 128).astype(ml_dtypes.float8_e4m3fn))

    in_maps = []
    for c in range(NCORES):
        us = [UPC * c + u for u in range(UPC)]
        wint = np.ascontiguousarray(
            (np.stack([w_in[g].T.reshape(KC, 128, N) for g in us])
             .transpose(2, 0, 1, 3).reshape(128, UPC * KC * N)
             * np.float32(W_SCALE)).astype(ml_dtypes.float8_e4m3fn))
        spc = np.ascontiguousarray(sp[:, c * 128:(c + 1) * 128])
        in_maps.append({"kspt": kspt, "wint": wint, "spc": spc})
    return in_maps


# ---------------------------------------------------------------- fallback
def _reference_host(sp, w_in, w_rec, unit_w, cw1, cb1, cw2, cb2):
    """Exact sequential evaluation (used only if any spike fires)."""
    m = np.float32(DT * TAU_MEM_INV)
    bsyn = np.float32(1.0 - DT * TAU_SYN_INV)
    outs = np.zeros((U, T, N), np.float32)
    for uu in range(U):
        z = np.zeros(N, np.float32)
        v = np.full(N, V_LEAK, np.float32)
        i = np.zeros(N, np.float32)
        for t in range(T):
            vd = v + m * ((V_LEAK - v) + i)
            idec = i * bsyn
            zn = (vd - V_TH > 0).astype(np.float32)
            vn = (1 - zn) * vd + zn * V_RESET
            i = idec + sp[t] @ w_in[uu].T + z @ w_rec[uu].T
            z, v = zn, vn
            outs[uu, t] = zn
    act = outs.mean(axis=1)
    h = np.maximum(act.reshape(-1) @ cw1.T + cb1, 0).astype(np.float32)
    probs = (1.0 / (1.0 + np.exp(-(h @ cw2.T + cb2)))).reshape(U, U)
    conn = (_u42() < probs).astype(np.float32)
    routed = np.einsum('ij,itn->tjn', conn, outs)
    applied = np.einsum('tjn,jnm->tjm', routed, unit_w)
    return (applied.mean(axis=1) + DIRECT_WEIGHT * sp).astype(np.float32)


# ---------------------------------------------------------------- entry
def kernel(input_spikes, w_in, w_rec, unit_w, cw1, cb1, cw2, cb2,
           **_unused):
    sp = np.ascontiguousarray(np.asarray(input_spikes, np.float32))
    w_in = np.asarray(w_in, np.float32)

    nc = _build_graph()
    in_maps = _prep_in_maps(sp, w_in)
    res = run_bass_kernel_spmd(nc, in_maps, core_ids=list(range(NCORES)))
    maxdrift = max(float(np.asarray(res.results[c]["zsum"]).reshape(-1)[0])
                   for c in range(NCORES))
    if maxdrift > DET_TH:
        # A spike may fire: the linearized fast path is invalid -> exact
        # host evaluation (never hit with the benchmark weight scales).
        return _reference_host(
            sp, w_in, np.asarray(w_rec, np.float32),
            np.asarray(unit_w, np.float32), np.asarray(cw1, np.float32),
            np.asarray(cb1, np.float32), np.asarray(cw2, np.float32),
            np.asarray(cb2, np.float32))
    out = np.concatenate(
        [np.asarray(res.results[c]["out"], np.float32)
         for c in range(NCORES)], axis=1)
    return np.ascontiguousarray(out)


if __name__ == "__main__":
    d = np.load("inputs.npz")
    got = kernel(**{k: d[k] for k in d.files})
    ref = np.load("golden.npy")
    err = np.abs(got - ref).max()
    denom = max(np.abs(ref).max(), 1e-9)
    print("abs err:", err, "rel:", err / denom)


# revision 6
# speedup vs baseline: 1.1446x; 1.1446x over previous
"""Trainium2 distributed kernel for the modular spiking-network module.

Model (reference semantics):
  1. 16 modular units, each a LIF recurrence over shared input spikes
     (T=100, N=1024) with per-unit input / recurrent [N,N] weights.
  2. Per-unit mean activity -> coordinator MLP -> sigmoid probs [16,16].
  3. Bernoulli routing matrix conn = (U42 < probs), U42 fixed uniform draws.
  4. routed = einsum('ij,itn->tjn', conn, outputs);
     applied = einsum('tjn,jnm->tjm', routed, unit_w);
     out = applied.mean(axis=1) + 1.5 * input_spikes.

Key structural facts exploited on-device:

  (a) While no neuron has spiked, the LIF dynamics are LINEAR, so the
      membrane drift is a causal filter of the input currents:
      v_dec(t) - V_LEAK = (K @ sp @ w_in[u].T)[t], with K a constant
      [T,T] kernel.  Folding Ksp = K @ sp on the host (tiny [T,N]
      precompute), the whole per-unit trajectory is ONE matmul
      drift_u = Ksp @ w_in[u].T, and a spike exists iff
      max(drift_u) > V_TH - V_LEAK = 15.

  (b) If no spike fires in ANY unit, every downstream stage is exactly
      zero: outputs == 0 -> activity == 0 -> routed == 0 -> applied == 0,
      and the module output is exactly DIRECT_WEIGHT * input_spikes.

So the device does exactly the irreducible work: it streams all 16
units' [N,N] input weights (fp8, units sharded 2-per-core), computes
each unit's full drift trajectory, and reduces it to a per-core
max-drift scalar.  Each core also emits its 128-column shard of the
residual output 1.5 * input_spikes.  The host ORs the 8 max-drift flags;
if any unit would spike (never observed with the benchmark weight
scales: max drift ~0.14 vs a 15.0 threshold gap, and we flag at 7.5 to
absorb fp8 rounding), it falls back to an exact sequential evaluation.
"""

import ml_dtypes
import numpy as np

import concourse.mybir as mybir
import concourse.tile as tile
from concourse import bacc
from concourse.bass_utils import run_bass_kernel_spmd

# ---------------------------------------------------------------- constants
T, U, N, H = 100, 16, 1024, 128
NCORES = 8
UPC = U // NCORES          # units per core
KC = N // 128              # 128-row chunks per [N,N] matrix
DT = 1e-3
TAU_MEM_INV = 1.0 / 20.0
TAU_SYN_INV = 1.0 / 10.0
V_LEAK = -70.0
V_TH = -55.0
V_RESET = -70.0
DIRECT_WEIGHT = 1.5
THRESH = np.float32(V_TH - V_LEAK)       # 15.0

# fp8 scaling: keep both operands well inside e4m3's normal range so the
# detection matmul loses no low-magnitude rows to denormal flush.
KSP_SCALE = 64.0           # Ksp entries <= ~0.25  -> <= 16
W_SCALE = 16.0             # |w_in| <= 1/32        -> <= 0.5
# device drift is scaled by KSP_SCALE*W_SCALE; flag at half the true
# threshold so fp8 rounding can never hide a real crossing.
DET_TH = 0.5 * float(THRESH) * KSP_SCALE * W_SCALE   # 7680.0

# jax.random.uniform(jax.random.key(42), (16, 16)) — fixed module constant
# (used only by the exact host fallback).
_U42_HEX = (
    "d010183f4043e03ee8e9203f80a1013c80838e3eb0304c3f7c265a3f6868763f"
    "5c332d3f406be33dc8f2fd3e3c2c3b3f9042423e40201e3ea0c31b3dccd2ab3e"
    "3ea26d3f10c61c3f5039fc3eb6da3b3fe0bf413e108f1d3e201c183d049aa83e"
    "86ec6f3f2883183fe855ec3eca682b3fc0c18f3d04c5c93e36d00e3f5c50b53e"
    "62a8733f4a50223f00bcd53bc4c28c3ebcd8493f20dc563f66c5683f1ae2143f"
    "c434d93e56781c3f6890fb3e40573e3f50b3653ea01e5c3ec0142f3e80e2933d"
    "b015c23e38f8073ffc2e943e36f2513fec45663f4c29093f00d2af3eae696a3f"
    "7478113f48c5ce3ee8550b3fd06da63e8afa633fa27d023f1ce2823e0e66473f"
    "12b04b3f06e0533ffa44633f1610023f7c59813e94e9453ff6614f3fe66d5a3f"
    "a07c703fc464243f2093423d3062b93ede997d3f3ec83f3f00ec6d3e1890483e"
    "486f023e409cbc3c7c069e3eb8cf5a3faa59713f540c263fe09f053de8e2a93e"
    "a0ae6d3f661b1f3fa4ddf43ea687303f386d153ea0886f3df403b33eb271773f"
    "d89e2a3f605a8a3d70fbcd3e5892093f5082ae3e9ee66a3f6c93113fe00ace3e"
    "f82d0a3fe0f2a03edcde643f32980d3f8012be3eec2a7a3fa66f303fc8a8133e"
    "a0005e3dacdcbf3ea6cd7b3f808a333fc80d0c3e800b053c8068803e5673443f"
    "d8f04c3f36e95d3fcedb7f3fd6b93b3ff8e34d3e48d70b3e005bf73bd0be8f3e"
    "d2404c3fac81543f6601653fb60e063fe036803eaa2e4c3ffe46543f7c98643f"
    "343e053f8c9e8c3ed4af403fdc624d3fc0f9563f8ac5613fb4a30f3f5cdda63e"
    "befb6a3ff4d4193fc46fff3e700f333f18ff293e7091c43d6c93fa3efaf6363f"
    "d853063ee017753d30b0a53eeec4693f70fd1f3fd029e73e0c792b3fc017d53d"
    "dc2ff23e96723e3f88e2423e1892343e5000b03d8490c03e78de0c3f24efab3e"
    "daa8673fbc12033fb036943e70f7583f8e2e7d3f9e9b363f98d2073ec051723d"
    "8c0ea53edac4693f964e1f3f748ae43e6853283f50c6e33d1c55e83e1481243f"
    "40e3ba3c48f88e3e2a37423f6e9e483f24cd5d3ffc6c773f9a2a223f20880a3d"
    "345bbb3e4ca0773f40b9233fa05b383dfcc6b73e2a2f7b3f00b03a3f182a663e"
    "98c67f3ed01d4c3eb8af2b3e50adc93d9c6bfd3e3ccb313f50d83c3ec069963d"
    "507bd23ec8de1e3f9c3de23e96912e3fc07e8c3d006adf3e54c0133f9c4fd63e"
    "f6f71a3f40aff23e7c163e3f204d423ec06c363e806bbc3d485ec73ee2e90b3f"
    "e8cdb63e507e7a3f024d383f28d4703ec0e9533ed892153e8008cb3c044c803e"
    "deda4c3ff020553f80d2663f3a39013f08db9d3e2246513f00166e3fbeb6103f"
    "30f7db3eba7f173fc4eec43e3e65083fa886b83ee008743f029f243fc06eb63c"
    "68da8f3e4a5e433f3c384a3f6af7583f426b7d3f8c54363f78ae003ec0634d3d"
)


def _u42() -> np.ndarray:
    return np.frombuffer(bytes.fromhex(_U42_HEX), dtype=np.float32).reshape(U, U)


def _kmat() -> np.ndarray:
    """K[t, r] = m * (a^(t-r) - b^(t-r)) / (a - b) for r < t, else 0 (f32)."""
    m = DT * TAU_MEM_INV
    a = 1.0 - m
    b = 1.0 - DT * TAU_SYN_INV
    d = np.arange(T, dtype=np.float64)
    coef = np.zeros(T, np.float64)
    coef[1:] = m * (a ** d[1:] - b ** d[1:]) / (a - b)
    idx = np.arange(T)
    K = coef[np.clip(idx[:, None] - idx[None, :], 0, T - 1)]
    K[idx[:, None] <= idx[None, :]] = 0.0
    return K.astype(np.float32)


# ---------------------------------------------------------------- graph
_GRAPH_CACHE = {}


def _build_graph():
    if "nc" in _GRAPH_CACHE:
        return _GRAPH_CACHE["nc"]

    f32 = mybir.dt.float32
    fp8 = mybir.dt.float8e4
    Alu = mybir.AluOpType

    nc = bacc.Bacc("TRN2", target_bir_lowering=False, debug=False,
                   num_devices=NCORES)

    # I/O (per-core shards / replicas)
    kspt_ext = nc.dram_tensor("kspt", [128, KC * 128], fp8,
                              kind="ExternalInput").ap()
    wint_ext = nc.dram_tensor("wint", [128, UPC * KC * N], fp8,
                              kind="ExternalInput").ap()
    spc_ext = nc.dram_tensor("spc", [T, 128], f32, kind="ExternalInput").ap()
    out_ext = nc.dram_tensor("out", [T, 128], f32, kind="ExternalOutput").ap()
    zsum_ext = nc.dram_tensor("zsum", [1, 1], f32, kind="ExternalOutput").ap()

    with tile.TileContext(nc) as tc:
        with (
            tc.tile_pool(name="wpool", bufs=4) as wpool,      # weight chunks
            tc.tile_pool(name="work", bufs=1) as work,        # persistents
            tc.tile_pool(name="ps", bufs=4, space="PSUM") as ps,
        ):
            # ---------- input DMAs, issue spread across all 5 engine
            # sequencers (a dma_start costs ~0.8us of issue time on its
            # engine; serializing them on Sync was the v2 bottleneck).
            # DMA packets drain round-robin across the 16 queues in ISSUE
            # order, so concurrent weight DMAs would interleave and all
            # finish at the end of the drain.  Instead: all weight chunks
            # go out serially from Activation (earliest engine to reach
            # main) in exactly the order the PE consumes them; the small
            # kspt/spc loads ride on GpSimd in parallel.
            kspt = work.tile([128, KC, 128], fp8)
            nc.gpsimd.dma_start(kspt[:],
                                kspt_ext.rearrange("p (k t) -> p k t", t=128))

            wch = {}
            for u in range(UPC):
                for h in range(2):
                    w = wpool.tile([128, 4, N], fp8, tag="wchunk",
                                   name=f"win_{u}_{h}")
                    nc.scalar.dma_start(
                        w[:], wint_ext[:, (u * KC + 4 * h) * N:
                                       (u * KC + 4 * h + 4) * N]
                        .rearrange("p (k n) -> p k n", k=4))
                    wch[(u, h)] = w

            spc = work.tile([T, 128], f32)
            nc.gpsimd.dma_start(spc[:], spc_ext)

            # ---------- residual output shard: 1.5 * spikes (independent)
            outc = work.tile([T, 128], f32)
            nc.vector.tensor_scalar(outc[:], spc[:], DIRECT_WEIGHT, None,
                                    Alu.mult)
            nc.sync.dma_start(out_ext, outc[:])

            # ---------- spike detection: drift_u = Ksp @ w_in[u].T
            # fp8 DoubleRow matmul chains, emitted in CHUNK-ARRIVAL order
            # (each 512KB chunk is fully consumed as soon as it lands, so
            # the PE pipelines behind the weight stream); the two PSUM
            # groups of a unit accumulate interleaved.
            mxc = work.tile([128, UPC * 2], f32)
            pv = {}
            for u in range(UPC):
                for mh in range(2):
                    pv[(u, mh)] = ps.tile([128, 512], f32, tag="ps",
                                          name=f"pv_{u}_{mh}")
            for u in range(UPC):
                for h in range(2):
                    for kk in range(2):
                        for mh in range(2):
                            jp = 2 * h + kk
                            nc.tensor.matmul(
                                pv[(u, mh)][:, :],
                                kspt[:, 2 * jp:2 * jp + 2, :],
                                wch[(u, h)][:, 2 * kk:2 * kk + 2,
                                            mh * 512:(mh + 1) * 512],
                                start=(jp == 0), stop=(jp == KC // 2 - 1),
                                perf_mode=mybir.MatmulPerfMode.DoubleRow)
                for mh in range(2):
                    g = 2 * u + mh
                    nc.vector.tensor_reduce(mxc[:, g:g + 1], pv[(u, mh)][:],
                                            mybir.AxisListType.X, Alu.max)

            # cross-partition max -> [1,1] scalar drift flag for the host;
            # reduce + store both live on GpSimd so the tail is short.
            zs = work.tile([1, 1], f32)
            nc.gpsimd.tensor_reduce(zs[:], mxc[:], mybir.AxisListType.XYZWC,
                                    Alu.max)
            nc.gpsimd.dma_start(zsum_ext, zs[:])

    nc.compile()
    _GRAPH_CACHE["nc"] = nc
    return nc


# ---------------------------------------------------------------- host prep
def _prep_in_maps(sp, w_in):
    K32 = _kmat()
    ksp = (K32.astype(np.float64) @ sp.astype(np.float64)) * KSP_SCALE  # [T,N]
    kspt3 = np.zeros((128, KC, 128), np.float32)
    kspt3[:, :, :T] = ksp.T.reshape(KC, 128, T).transpose(1, 0, 2)
    kspt = np.ascontiguousarray(
        kspt3.reshape(128, KC * 128).astype(ml_dtypes.float8_e4m3fn))

    in_maps = []
    for c in range(NCORES):
        us = [UPC * c + u for u in range(UPC)]
        wint = np.ascontiguousarray(
            (np.stack([w_in[g].T.reshape(KC, 128, N) for g in us])
             .transpose(2, 0, 1, 3).reshape(128, UPC * KC * N)
             * np.float32(W_SCALE)).astype(ml_dtypes.float8_e4m3fn))
        spc = np.ascontiguousarray(sp[:, c * 128:(c + 1) * 128])
        in_maps.append({"kspt": kspt, "wint": wint, "spc": spc})
    return in_maps


# ---------------------------------------------------------------- fallback
def _reference_host(sp, w_in, w_rec, unit_w, cw1, cb1, cw2, cb2):
    """Exact sequential evaluation (used only if any spike fires)."""
    m = np.float32(DT * TAU_MEM_INV)
    bsyn = np.float32(1.0 - DT * TAU_SYN_INV)
    outs = np.zeros((U, T, N), np.float32)
    for uu in range(U):
        z = np.zeros(N, np.float32)
        v = np.full(N, V_LEAK, np.float32)
        i = np.zeros(N, np.float32)
        for t in range(T):
            vd = v + m * ((V_LEAK - v) + i)
            idec = i * bsyn
            zn = (vd - V_TH > 0).astype(np.float32)
            vn = (1 - zn) * vd + zn * V_RESET
            i = idec + sp[t] @ w_in[uu].T + z @ w_rec[uu].T
            z, v = zn, vn
            outs[uu, t] = zn
    act = outs.mean(axis=1)
    h = np.maximum(act.reshape(-1) @ cw1.T + cb1, 0).astype(np.float32)
    probs = (1.0 / (1.0 + np.exp(-(h @ cw2.T + cb2)))).reshape(U, U)
    conn = (_u42() < probs).astype(np.float32)
    routed = np.einsum('ij,itn->tjn', conn, outs)
    applied = np.einsum('tjn,jnm->tjm', routed, unit_w)
    return (applied.mean(axis=1) + DIRECT_WEIGHT * sp).astype(np.float32)


# ---------------------------------------------------------------- entry
def kernel(input_spikes, w_in, w_rec, unit_w, cw1, cb1, cw2, cb2,
           **_unused):
    sp = np.ascontiguousarray(np.asarray(input_spikes, np.float32))
    w_in = np.asarray(w_in, np.float32)

    nc = _build_graph()
    in_maps = _prep_in_maps(sp, w_in)
    res = run_bass_kernel_spmd(nc, in_maps, core_ids=list(range(NCORES)))
    maxdrift = max(float(np.asarray(res.results[c]["zsum"]).reshape(-1)[0])
                   for c in range(NCORES))
    if maxdrift > DET_TH:
        # A spike may fire: the linearized fast path is invalid -> exact
        # host evaluation (never hit with the benchmark weight scales).
        return _reference_host(
            sp, w_in, np.asarray(w_rec, np.float32),
            np.asarray(unit_w, np.float32), np.asarray(cw1, np.float32),
            np.asarray(cb1, np.float32), np.asarray(cw2, np.float32),
            np.asarray(cb2, np.float32))
    out = np.concatenate(
        [np.asarray(res.results[c]["out"], np.float32)
         for c in range(NCORES)], axis=1)
    return np.ascontiguousarray(out)


if __name__ == "__main__":
    d = np.load("inputs.npz")
    got = kernel(**{k: d[k] for k in d.files})
    ref = np.load("golden.npy")
    err = np.abs(got - ref).max()
    denom = max(np.abs(ref).max(), 1e-9)
    print("abs err:", err, "rel:", err / denom)
